# revision 28
# baseline (speedup 1.0000x reference)
"""ConvLSTM (B=4, T=8, C=HID=256, H=W=32, 3x3 SAME convs) on 8 TRN2 NeuronCores.

Sharding: data-parallel over batch (4) x spatial halves of H (2) = 8 cores.
Each core owns 16 rows and computes exactly 16 rows per step; the single
boundary row of h it needs from its partner is exchanged each step with a
2-core CC AllReduce (sum of both boundary rows staged in DRAM, partner row
recovered by subtracting our own contribution — keeps the instruction
stream SPMD-identical). Upper halves are row-flipped host-side (with
dy-flipped kernels) so all 8 cores run the same program.

Compute: conv as 36 PE matmuls per step per octile (2 convs x 2 ic-tiles x
9 taps) at bf16 (fast-weight-load keeps LDWEIGHTS hidden), fp32 PSUM
accumulation. Gates: sigmoid on ScalarE (bias fused), relu+bias on
VectorE. State update on VectorE. x-conv matmuls are issued before h-conv
matmuls so the PE stays busy while the halo row is in flight.
"""
import numpy as np
import ml_dtypes
from contextlib import ExitStack

import concourse.bass as bass
import concourse.tile as tile
from concourse import bacc, mybir
from concourse.bass_utils import run_bass_kernel_spmd

BF16 = mybir.dt.bfloat16
NPBF = ml_dtypes.bfloat16
F32 = mybir.dt.float32
AF = mybir.ActivationFunctionType
ALU = mybir.AluOpType

N_CORES = 8
T = 8
ROWS = 19          # p=0 zero row, p=1..17 rows 0..16 (16 owned + halo row
                   # 16), p=18 second x-halo row 17 (used by t=0 only)
WC = 34            # padded width
PLANE = ROWS * WC  # 646
CROWS = 17         # c rows: 16 owned + row 16 (written at t=0 only)
CPL = CROWS * 32
R = 16             # rows computed per step (t=0: 17, see below)
N = R * 32         # 512: matmul free dim / psum bank width

PAIRS = [[0, 1], [2, 3], [4, 5], [6, 7]]

_cache = {}

# tap order: dy=1 row first so the start=True matmul is always full-width
# (the dy=0 tap of output row 1 reads the permanent zero row — dropped)
KORD = [3, 4, 5, 0, 1, 2, 6, 7, 8]


def _build_nc():
    nc = bacc.Bacc("TRN2", target_bir_lowering=False, debug=False,
                   num_devices=N_CORES)
    x_d = nc.dram_tensor("xb", [T, 128, 2 * PLANE], BF16, kind="ExternalInput").ap()
    w_d = nc.dram_tensor("w", [36, 128, 1024], BF16, kind="ExternalInput").ap()
    b_d = nc.dram_tensor("bias", [128, 8], F32, kind="ExternalInput").ap()
    z_d = nc.dram_tensor("hz", [128, 2 * PLANE], BF16, kind="ExternalInput").ap()
    out_d = nc.dram_tensor("hout", [2, 128, 512], BF16, kind="ExternalOutput").ap()
    ccb = nc.dram_tensor("ccb", [128, 64], F32).ap()  # boundary-row mailbox

    with tile.TileContext(nc) as tc, ExitStack() as ctx:
        wp = ctx.enter_context(tc.tile_pool(name="wp", bufs=1))
        xp = ctx.enter_context(tc.tile_pool(name="xp", bufs=2))
        hp = ctx.enter_context(tc.tile_pool(name="hp", bufs=1))
        cp = ctx.enter_context(tc.tile_pool(name="cp", bufs=1))
        bp = ctx.enter_context(tc.tile_pool(name="bp", bufs=1))
        gp = ctx.enter_context(tc.tile_pool(name="gp", bufs=10))
        tp = ctx.enter_context(tc.tile_pool(name="tp", bufs=3))
        ep = ctx.enter_context(tc.tile_pool(name="ep", bufs=4))
        pp = ctx.enter_context(tc.tile_pool(name="pp", bufs=8, space="PSUM"))

        bt = bp.tile([128, 8], F32, tag="bias")
        nc.sync.dma_start(bt[:], b_d[:])

        ha = hp.tile([128, 2 * PLANE], BF16, tag="ha")
        hb = hp.tile([128, 2 * PLANE], BF16, tag="hb")
        ct = cp.tile([128, 2 * CPL], F32, tag="c")
        nc.vector.memset(ct[:], 0.0)
        hbufs = [ha, hb]

        # x and the h zero-fills ride the gpsimd (SWDGE) queue so they never
        # wait behind the 9.4MB weight stream on the sync (HWDGE) queue.
        # t=0 chunk-1 (rows 0..8) reads x rows 0..10 = first 374 cols of
        # each plane: load those first, on the scalar HWDGE queue (idle at
        # start) so they run parallel to the weight stream on sync.
        x0 = xp.tile([128, 2 * PLANE], BF16, tag="x")
        nc.scalar.dma_start(x0[:, :374], x_d[0][:, :374])
        nc.scalar.dma_start(x0[:, PLANE:PLANE + 374],
                            x_d[0][:, PLANE:PLANE + 374])
        nc.gpsimd.dma_start(x0[:, 374:PLANE], x_d[0][:, 374:PLANE])
        nc.gpsimd.dma_start(x0[:, PLANE + 374:], x_d[0][:, PLANE + 374:])
        nc.gpsimd.dma_start(hb[:], z_d[:])
        nc.gpsimd.dma_start(ha[:], z_d[:])

        # One tile per weight slice so a matmul only waits on the slice it
        # reads. Full [128,1024] tiles: the DMA source is then fully
        # contiguous per partition (fast path), unlike column-split slices.
        # DMAs are issued in first-use (KORD, it-major) order so the t=0
        # matmuls start sooner.
        ws = [wp.tile([128, 1024], BF16, tag=f"w{j}", name=f"w{j}")
              for j in range(36)]
        JORD = [it * 9 + k for it in range(2) for k in KORD]
        for j in JORD:
            nc.sync.dma_start(ws[j][:], w_d[j])
        for j in JORD:
            nc.sync.dma_start(ws[18 + j][:], w_d[18 + j])

        def wslice(j, o):
            return ws[j][:, o * 128:(o + 1) * 128]

        next_x = x0
        for t in range(T):
            h_in, h_out = hbufs[t % 2], hbufs[(t + 1) % 2]
            xt = next_x
            if t < T - 1:
                # prefetch x for t+1 now, BEFORE the exchange ops put a
                # cc_sem wait on the gpsimd queue
                next_x = xp.tile([128, 2 * PLANE], BF16, tag="x")
                nc.gpsimd.dma_start(next_x[:], x_d[t + 1])
            xv = xt[:].rearrange("p (i r c) -> p i r c", i=2, r=ROWS, c=WC)
            hv = h_in[:].rearrange("p (i r c) -> p i r c", i=2, r=ROWS, c=WC)
            hov = h_out[:].rearrange("p (i r c) -> p i r c", i=2, r=ROWS, c=WC)

            # t=0: h_0 == 0, so skip all h-conv matmuls; f-gate is unused
            # (f*c_0 == 0), so skip its two octiles entirely. t=0 computes
            # 17 rows (one extra, from the 2-row x halo) so t=1's h-conv
            # needs no exchange — the first real exchange (t=1) then has a
            # full extra step to hide its latency.
            # octile order (host-reordered): 0,1=i  2,3=o  4,5=g  6,7=f
            if t == 0:
                octs = [0, 1, 2, 3, 4, 5]
                chunks = [(1, 9), (10, 8)]
            elif t == T - 1:
                # split the last step asymmetrically: the final serial
                # vector chain then runs on a quarter-size chunk
                octs = list(range(8))
                chunks = [(1, 12), (13, 4)]
            else:
                octs = list(range(8))
                chunks = [(1, R)]

            def emit_mm(ps, src, j, o, it, k, q, r, start, stop):
                dy, dx = k // 3, k % 3
                if q == 1 and dy == 0:
                    # dy=0 tap of output row 1 reads the permanent zero
                    # row -> drop that row from the MM
                    nc.tensor.matmul(
                        ps[:, 32:], wslice(j, o),
                        src[:, it, 1: r, dx: dx + 32],
                        start=start, stop=stop, skip_group_check=True)
                else:
                    nc.tensor.matmul(
                        ps[:], wslice(j, o),
                        src[:, it, q + dy - 1: q + dy - 1 + r, dx: dx + 32],
                        start=start, stop=stop, skip_group_check=True)

            for (q, r) in chunks:
                n = r * 32
                ps_tiles = {}
                # x-conv half first: independent of the recurrence and of
                # the halo row in flight.  At t=0 the weight slices are
                # still streaming in from HBM, so iterate j-major to
                # consume them in arrival order.
                if t == 0:
                    for o in octs:
                        ps_tiles[o] = pp.tile([128, n], F32, tag="ps",
                                              name=f"ps{o}")
                    for it in range(2):
                        for k in KORD:
                            j = it * 9 + k
                            for o in octs:
                                emit_mm(ps_tiles[o], xv, j, o, it, k, q, r,
                                        start=(it == 0 and k == KORD[0]),
                                        stop=(it == 1 and k == KORD[-1]))
                else:
                    for o in octs:
                        ps = pp.tile([128, n], F32, tag="ps")
                        ps_tiles[o] = ps
                        for it in range(2):
                            for k in KORD:
                                emit_mm(ps, xv, it * 9 + k, o, it, k, q, r,
                                        start=(it == 0 and k == KORD[0]),
                                        stop=False)
                    for o in octs:
                        ps = ps_tiles[o]
                        for it in range(2):
                            for k in KORD:
                                emit_mm(ps, hv, 18 + it * 9 + k, o, it, k,
                                        q, r, start=False,
                                        stop=(it == 1 and k == KORD[-1]))
                gts = {}
                for o in octs:
                    gt = gp.tile([128, n], F32, tag="g")
                    gts[o] = gt
                    if o < 4 or o >= 6:  # i, o, f -> sigmoid; g -> relu
                        nc.scalar.activation(gt[:], ps_tiles[o][:],
                                             AF.Sigmoid, bias=bt[:, o:o + 1])
                    else:
                        nc.vector.tensor_scalar(gt[:], ps_tiles[o][:],
                                                bt[:, o:o + 1], 0.0,
                                                ALU.add, ALU.max)
                if 0 < t < T - 1:
                    stage = ep.tile([128, 64], F32, tag="st", name="stage")
                else:
                    stage = None
                for hi in range(2):
                    gi, go, gg = gts[0 + hi], gts[2 + hi], gts[4 + hi]
                    c0 = hi * CPL + (q - 1) * 32
                    cs = ct[:, c0: c0 + n]
                    if t == 0:
                        nc.vector.tensor_mul(cs, gi[:], gg[:])
                    else:
                        gf = gts[6 + hi]
                        nc.vector.tensor_mul(gg[:], gi[:], gg[:])
                        nc.vector.tensor_mul(cs, gf[:], cs)
                        nc.vector.tensor_add(cs, cs, gg[:])
                    # h = relu(c) * o fused into one DVE op
                    nc.vector.scalar_tensor_tensor(
                        hov[:, hi, q: q + r, 1: 33], cs, 0.0, go[:],
                        ALU.max, ALU.mult)
                    if stage is not None:
                        # boundary row (local row 15) fp32 for the exchange
                        nc.vector.scalar_tensor_tensor(
                            stage[:, hi * 32:(hi + 1) * 32],
                            ct[:, c0 + 480: c0 + 512], 0.0,
                            go[:, 480:512], ALU.max, ALU.mult)
                    if t == T - 1:
                        nc.sync.dma_start(
                            out_d[hi][:, (q - 1) * 32: (q - 1) * 32 + n],
                            hov[:, hi, q: q + r, 1: 33])

            if 0 < t < T - 1:
                # pair-wise exchange of the boundary row: AllReduce(add) of
                # both cores' rows, partner row = sum - ours. The tile
                # framework tracks the DRAM mailbox, so it orders
                # DMA-out -> AllReduce -> DMA-in (and the WAR on reuse).
                nc.sync.dma_start(ccb[:], stage[:])
                nc.gpsimd.collective_compute(
                    "AllReduce", ALU.add, replica_groups=PAIRS,
                    ins=[ccb[:].opt()], outs=[ccb[:].opt()],
                )
                sumt = ep.tile([128, 64], F32, tag="sm")
                nc.gpsimd.dma_start(sumt[:], ccb[:])
                for hi in range(2):
                    nc.vector.tensor_sub(hov[:, hi, 17: 18, 1: 33],
                                         sumt[:, hi * 32:(hi + 1) * 32],
                                         stage[:, hi * 32:(hi + 1) * 32])

    nc.compile()
    return nc


GATE_PERM = [0, 2, 3, 1]  # reorder [i, f, o, g] -> [i, o, g, f]


def _prep_weights(wx, wh, flip):
    ws = np.stack([np.asarray(wx), np.asarray(wh)])  # [2, 1024, 256, 3, 3]
    if flip:
        ws = ws[:, :, :, ::-1, :]
    # [cv, gate, ht, ch, it, ic, dy, dx] -> [cv, it, dy, dx, ic, gate, ht, ch]
    ws = ws.reshape(2, 4, 2, 128, 2, 128, 3, 3)[:, GATE_PERM]
    ws = ws.transpose(0, 4, 6, 7, 5, 1, 2, 3)
    return np.ascontiguousarray(ws.reshape(36, 128, 1024)).astype(NPBF)


def _prep_x(xb, flip):
    # xb: [T, 256, 32, 32] for one batch element -> [T, 128, 2*PLANE]
    # rows 0..17 of the (possibly flipped) image: 16 owned + 2 halo rows
    # (row 17 is only read by t=0, which computes 17 output rows).
    xc = np.asarray(xb)
    if flip:
        xc = xc[:, :, ::-1, :]
    buf = np.zeros((T, 2, 128, ROWS, WC), dtype=np.float32)
    for it in range(2):
        buf[:, it, :, 1:19, 1:33] = xc[:, it * 128:(it + 1) * 128, 0:18, :]
    buf = buf.reshape(T, 2, 128, PLANE).transpose(0, 2, 1, 3)
    return np.ascontiguousarray(buf).reshape(T, 128, 2 * PLANE).astype(NPBF)


def kernel(x, wx, wh, bh):
    x = np.asarray(x, dtype=np.float32)
    B = x.shape[0]
    bias = np.ascontiguousarray(
        np.asarray(bh, dtype=np.float32).reshape(4, 2, 128)[GATE_PERM]
        .transpose(2, 0, 1).reshape(128, 8))

    w_lo = _prep_weights(wx, wh, flip=False)
    w_hi = _prep_weights(wx, wh, flip=True)

    in_maps = []
    for c in range(N_CORES):
        b, half = c // 2, c % 2
        in_maps.append({
            "xb": _prep_x(x[b], flip=bool(half)),
            "w": w_hi if half else w_lo,
            "bias": bias,
            "hz": np.zeros((128, 2 * PLANE), dtype=NPBF),
        })

    if "nc" not in _cache:
        _cache["nc"] = _build_nc()
    nc = _cache["nc"]

    res = run_bass_kernel_spmd(nc, in_maps, core_ids=list(range(N_CORES)))
    _cache["last_results"] = res

    out = np.zeros((B, 256, 32, 32), dtype=np.float32)
    for c in range(N_CORES):
        b, half = c // 2, c % 2
        h = res.results[c]["hout"].astype(np.float32).reshape(2, 128, 16, 32)
        h = np.concatenate([h[0], h[1]], axis=0)  # [256, 16, 32]
        if half:
            out[b, :, 16:32, :] = h[:, ::-1, :]
        else:
            out[b, :, 0:16, :] = h
    return out
